# revision 16
# baseline (speedup 1.0000x reference)
"""CenterLoss Trainium2 kernel.

loss = (sum_i clamp(||x_i - centers[labels_i]||^2, 1e-12, 1e12)
        + BS*(C_OUT-1)*1e-12) / BS

Masking the full [BS, C_OUT] distance matrix keeps exactly one distance
per row; the other BS*(C_OUT-1) clamped zeros are a host-side constant.

Data-parallel over batch across 8 NeuronCores, centers replicated.  Each
core gathers its 1024 label rows with indirect_dma_start (the resident
SWDGE indirect1d path, int32 indices), which -- unlike the mlp-library
dma_gather ucode -- needs no load_library, so the ~11us ucode IRAM load
disappears from the critical path and the gather can start as soon as
the 4KB label DMA lands (~2us into the kernel).

HW indirect1d semantics (measured): ONE index per destination
partition, gathering that partition's whole free extent contiguously
from src[idx[p]] -- the bass_interp multi-index-per-partition model
diverges from hardware.  So the 1024-row gather is RPP=8 calls, each
[128,1] indices -> [128, D] rows.  Each call gets its own completion
semaphore: per-engine sem increments make shared-sem thresholds racy.

Row r of the shard lives at SBUF [r // RPP, r % RPP]: partition-major
blocks make the x load a plain contiguous [128, 2KB] HWDGE copy and the
label load a [128, 32B] copy (both just reshape views on the host).
Descriptor generation of call k+1 overlaps the SDMA transfer of call k;
DVE consumes the gathered rows in two 4-column chunks (sub, square,
row-reduce), clamps, and the host sums the [128, RPP] partials in f64.

Host-side input prep is limited to sharding/replication, the int64 ->
int32 label cast, and reshape views; all data math runs on device.
Raw Bass blocks (no TileContext) avoid the Tile kernel-tail barrier.
"""

import numpy as np

BS, C_OUT, D = 8192, 50000, 64
N_CORES = 8
ROWS = BS // N_CORES  # rows per core
P = 128  # SBUF partitions
RPP = ROWS // P  # rows per partition (row r lives at [r // RPP, r % RPP])
CLAMP_MIN, CLAMP_MAX = 1e-12, 1e12
CHUNKS = (6, 1, 1)  # DVE consumption chunks in columns; sums to RPP

_CACHE = {}


def _build_program():
    import concourse.bacc as bacc
    import concourse.bass as bass
    import concourse.mybir as mybir

    nc = bacc.Bacc(
        "TRN2", target_bir_lowering=False, debug=False, num_devices=N_CORES
    )

    f32 = mybir.dt.float32
    i32 = mybir.dt.int32

    x_d = nc.dram_tensor("x", [P, RPP * D], f32, kind="ExternalInput")
    lab_d = nc.dram_tensor("labels_pn", [P, RPP], i32, kind="ExternalInput")
    cen_d = nc.dram_tensor("centers", [C_OUT, D], f32, kind="ExternalInput")
    out_d = nc.dram_tensor("out", [P, RPP], f32, kind="ExternalOutput")

    from contextlib import ExitStack
    with ExitStack() as ctx:
        x_t = ctx.enter_context(nc.sbuf_tensor("x_t", [P, RPP * D], f32))
        idx_t = ctx.enter_context(nc.sbuf_tensor("idx_t", [P, RPP], i32))
        g_t = ctx.enter_context(nc.sbuf_tensor("g_t", [P, RPP * D], f32))
        d_t = ctx.enter_context(nc.sbuf_tensor("d_t", [P, RPP * D], f32))
        s_t = ctx.enter_context(nc.sbuf_tensor("s_t", [P, RPP], f32))
        cl_t = ctx.enter_context(nc.sbuf_tensor("cl_t", [P, RPP], f32))
        s_lab = ctx.enter_context(nc.semaphore("s_lab"))
        s_x = ctx.enter_context(nc.semaphore("s_x"))
        # one sem per DVE chunk: calls of a chunk share it; the wait
        # threshold 16*len(chunk calls) is reached only when every call is
        # fully drained (each call contributes exactly 16 increments)
        s_g = [ctx.enter_context(nc.semaphore(f"s_g{i}")) for i in range(len(CHUNKS))]
        s_v = ctx.enter_context(nc.semaphore("s_v"))
        s_dve = ctx.enter_context(nc.semaphore("s_dve"))
        block = ctx.enter_context(nc.Block())

        @block.sync
        def _(sync: bass.BassEngine):
            # labels first: they gate the gather (critical path)
            sync.dma_start(out=idx_t[:], in_=lab_d[:]).then_inc(s_lab, 16)
            sync.dma_start(out=x_t[:], in_=x_d[:]).then_inc(s_x, 16)
            # writeback; no completion wait -- NEFF epilogue drains HWDGE
            sync.wait_ge(s_dve, 1)
            # reuse s_lab for the writeback completion (nothing waits on it;
            # walrus just needs a sem update on every DMA)
            sync.dma_start(out=out_d[:], in_=cl_t[:]).then_inc(s_lab, 16)

        @block.gpsimd
        def _(gpsimd: bass.BassGpSimd):
            gpsimd.wait_ge(s_lab, 16)
            n = 0
            for q, ncols in enumerate(CHUNKS):
                for _ in range(ncols):
                    gpsimd.indirect_dma_start(
                        out=g_t[:, n * D : (n + 1) * D],
                        out_offset=None,
                        in_=cen_d[:],
                        in_offset=bass.IndirectOffsetOnAxis(
                            ap=idx_t[:, n : n + 1], axis=0
                        ),
                    ).then_inc(s_g[q], 16)
                    n += 1

        @block.vector
        def _(vector: bass.BassEngine):
            # DVE has no same-engine interlock: s_v counts completions
            nv = 0
            x_v = x_t[:].rearrange("p (n m) -> p n m", m=D)
            d_v = d_t[:].rearrange("p (n m) -> p n m", m=D)

            vector.wait_ge(s_x, 16)
            c0 = 0
            for q, ncols in enumerate(CHUNKS):
                cs = slice(c0, c0 + ncols)
                fs = slice(c0 * D, (c0 + ncols) * D)
                vector.wait_ge(s_g[q], 16 * ncols)
                vector.tensor_tensor(
                    out=d_t[:, fs], in0=x_t[:, fs], in1=g_t[:, fs],
                    op=mybir.AluOpType.subtract,
                ).then_inc(s_v, 1)
                nv += 1
                vector.wait_ge(s_v, nv)
                vector.tensor_tensor(
                    out=d_t[:, fs], in0=d_t[:, fs], in1=d_t[:, fs],
                    op=mybir.AluOpType.mult,
                ).then_inc(s_v, 1)
                nv += 1
                vector.wait_ge(s_v, nv)
                vector.reduce_sum(
                    out=s_t[:, cs], in_=d_v[:, cs, :], axis=mybir.AxisListType.X
                ).then_inc(s_v, 1)
                nv += 1
                c0 += ncols

            vector.wait_ge(s_v, nv)
            vector.tensor_scalar(
                out=cl_t[:],
                in0=s_t[:],
                scalar1=CLAMP_MIN,
                scalar2=CLAMP_MAX,
                op0=mybir.AluOpType.max,
                op1=mybir.AluOpType.min,
            ).then_inc(s_dve, 1)

    nc.compile()
    return nc


def _get_program():
    if "nc" not in _CACHE:
        _CACHE["nc"] = _build_program()
    return _CACHE["nc"]


def kernel(x, labels, centers, trace=False):
    from concourse.bass_utils import run_bass_kernel_spmd

    nc = _get_program()

    x = np.ascontiguousarray(np.asarray(x, dtype=np.float32))
    labels_i32 = np.ascontiguousarray(np.asarray(labels, dtype=np.int32))
    centers = np.ascontiguousarray(np.asarray(centers, dtype=np.float32))

    in_maps = []
    for i in range(N_CORES):
        in_maps.append(
            {
                "x": x[i * ROWS : (i + 1) * ROWS].reshape(P, RPP * D),
                "labels_pn": labels_i32[i * ROWS : (i + 1) * ROWS].reshape(P, RPP),
                "centers": centers,
            }
        )

    res = run_bass_kernel_spmd(
        nc, in_maps, core_ids=list(range(N_CORES)), trace=trace
    )

    total = np.float64(0.0)
    for r in res.results:
        total += np.sum(r["out"], dtype=np.float64)
    # masked-out entries: BS*(C_OUT-1) zeros, each clamped to 1e-12
    total += np.float64(BS) * np.float64(C_OUT - 1) * 1e-12
    loss = np.float32(total / BS)

    if trace:
        _CACHE["last_exec_time_ns"] = res.exec_time_ns
        _CACHE["last_results"] = res
    return np.array(loss, dtype=np.float32)


# revision 19
# speedup vs baseline: 1.1088x; 1.1088x over previous
"""CenterLoss Trainium2 kernel.

loss = (sum_i clamp(||x_i - centers[labels_i]||^2, 1e-12, 1e12)
        + BS*(C_OUT-1)*1e-12) / BS

Masking the full [BS, C_OUT] distance matrix keeps exactly one distance
per row; the other BS*(C_OUT-1) clamped zeros are a host-side constant.

Data-parallel over batch across 8 NeuronCores, centers replicated.  Each
core gathers its 1024 label rows with indirect_dma_start (the resident
SWDGE indirect1d path, int32 indices), which -- unlike the mlp-library
dma_gather ucode -- needs no load_library, so the ~11us ucode IRAM load
disappears from the critical path and the gather can start as soon as
the 4KB label DMA lands (~2us into the kernel).

HW indirect1d semantics (measured): ONE index per destination
partition, gathering that partition's whole free extent contiguously
from src[idx[p]] -- the bass_interp multi-index-per-partition model
diverges from hardware.  So the 1024-row gather is RPP=8 calls, each
[128,1] indices -> [128, D] rows.  Each call gets its own completion
semaphore: per-engine sem increments make shared-sem thresholds racy.

Row r of the shard lives at SBUF [r // RPP, r % RPP]: partition-major
blocks make the x load a plain contiguous [128, 2KB] HWDGE copy and the
label load a [128, 32B] copy (both just reshape views on the host).
Descriptor generation of call k+1 overlaps the SDMA transfer of call k;
DVE consumes the gathered rows in two 4-column chunks (sub, square,
row-reduce), clamps, and the host sums the [128, RPP] partials in f64.

Host-side input prep is limited to sharding/replication, the int64 ->
int32 label cast, and reshape views; all data math runs on device.
Raw Bass blocks (no TileContext) avoid the Tile kernel-tail barrier.
"""

import numpy as np

BS, C_OUT, D = 8192, 50000, 64
N_CORES = 8
ROWS = BS // N_CORES  # rows per core
P = 128  # SBUF partitions
RPP = ROWS // P  # rows per partition (row r lives at [r // RPP, r % RPP])
CLAMP_MIN, CLAMP_MAX = 1e-12, 1e12
CHUNKS = (6, 1, 1)  # DVE consumption chunks in columns; sums to RPP

_CACHE = {}


def _build_program():
    import concourse.bacc as bacc
    import concourse.bass as bass
    import concourse.mybir as mybir

    nc = bacc.Bacc(
        "TRN2", target_bir_lowering=False, debug=False, num_devices=N_CORES
    )

    f32 = mybir.dt.float32
    i32 = mybir.dt.int32

    x_d = nc.dram_tensor("x", [P, RPP * D], f32, kind="ExternalInput")
    lab_d = nc.dram_tensor("labels_pn", [P, RPP], i32, kind="ExternalInput")
    cen_d = nc.dram_tensor("centers", [C_OUT, D], f32, kind="ExternalInput")
    out_d = nc.dram_tensor("out", [P, RPP], f32, kind="ExternalOutput")

    from contextlib import ExitStack
    with ExitStack() as ctx:
        x_t = ctx.enter_context(nc.sbuf_tensor("x_t", [P, RPP * D], f32))
        idx_t = ctx.enter_context(nc.sbuf_tensor("idx_t", [P, RPP], i32))
        g_t = ctx.enter_context(nc.sbuf_tensor("g_t", [P, RPP * D], f32))
        d_t = ctx.enter_context(nc.sbuf_tensor("d_t", [P, RPP * D], f32))
        s_t = ctx.enter_context(nc.sbuf_tensor("s_t", [P, RPP], f32))
        cl_t = ctx.enter_context(nc.sbuf_tensor("cl_t", [P, RPP], f32))
        s_lab = ctx.enter_context(nc.semaphore("s_lab"))
        s_x = ctx.enter_context(nc.semaphore("s_x"))
        s_g = [ctx.enter_context(nc.semaphore(f"s_g{i}")) for i in range(RPP)]
        s_v = ctx.enter_context(nc.semaphore("s_v"))
        s_dve = ctx.enter_context(nc.semaphore("s_dve"))
        block = ctx.enter_context(nc.Block())

        @block.sync
        def _(sync: bass.BassEngine):
            # labels first: they gate the gather (critical path)
            sync.dma_start(out=idx_t[:], in_=lab_d[:]).then_inc(s_lab, 16)
            sync.dma_start(out=x_t[:], in_=x_d[:]).then_inc(s_x, 16)
            # writeback; no completion wait -- NEFF epilogue drains HWDGE
            sync.wait_ge(s_dve, 1)
            # reuse s_lab for the writeback completion (nothing waits on it;
            # walrus just needs a sem update on every DMA)
            sync.dma_start(out=out_d[:], in_=cl_t[:]).then_inc(s_lab, 16)

        @block.gpsimd
        def _(gpsimd: bass.BassGpSimd):
            gpsimd.wait_ge(s_lab, 16)
            for n in range(RPP):
                gpsimd.indirect_dma_start(
                    out=g_t[:, n * D : (n + 1) * D],
                    out_offset=None,
                    in_=cen_d[:],
                    in_offset=bass.IndirectOffsetOnAxis(
                        ap=idx_t[:, n : n + 1], axis=0
                    ),
                ).then_inc(s_g[n], 16)

        @block.vector
        def _(vector: bass.BassEngine):
            # DVE has no same-engine interlock: s_v counts completions
            nv = 0
            x_v = x_t[:].rearrange("p (n m) -> p n m", m=D)
            d_v = d_t[:].rearrange("p (n m) -> p n m", m=D)

            vector.wait_ge(s_x, 16)
            c0 = 0
            for ncols in CHUNKS:
                cs = slice(c0, c0 + ncols)
                fs = slice(c0 * D, (c0 + ncols) * D)
                for n in range(c0, c0 + ncols):
                    vector.wait_ge(s_g[n], 16)
                vector.tensor_tensor(
                    out=d_t[:, fs], in0=x_t[:, fs], in1=g_t[:, fs],
                    op=mybir.AluOpType.subtract,
                ).then_inc(s_v, 1)
                nv += 1
                vector.wait_ge(s_v, nv)
                vector.tensor_tensor(
                    out=d_t[:, fs], in0=d_t[:, fs], in1=d_t[:, fs],
                    op=mybir.AluOpType.mult,
                ).then_inc(s_v, 1)
                nv += 1
                vector.wait_ge(s_v, nv)
                vector.reduce_sum(
                    out=s_t[:, cs], in_=d_v[:, cs, :], axis=mybir.AxisListType.X
                ).then_inc(s_v, 1)
                nv += 1
                c0 += ncols

            vector.wait_ge(s_v, nv)
            vector.tensor_scalar(
                out=cl_t[:],
                in0=s_t[:],
                scalar1=CLAMP_MIN,
                scalar2=CLAMP_MAX,
                op0=mybir.AluOpType.max,
                op1=mybir.AluOpType.min,
            ).then_inc(s_dve, 1)

    nc.compile()
    return nc


def _get_program():
    if "nc" not in _CACHE:
        _CACHE["nc"] = _build_program()
    return _CACHE["nc"]


def kernel(x, labels, centers, trace=False):
    from concourse.bass_utils import run_bass_kernel_spmd

    nc = _get_program()

    x = np.ascontiguousarray(np.asarray(x, dtype=np.float32))
    labels_i32 = np.ascontiguousarray(np.asarray(labels, dtype=np.int32))
    centers = np.ascontiguousarray(np.asarray(centers, dtype=np.float32))

    in_maps = []
    for i in range(N_CORES):
        in_maps.append(
            {
                "x": x[i * ROWS : (i + 1) * ROWS].reshape(P, RPP * D),
                "labels_pn": labels_i32[i * ROWS : (i + 1) * ROWS].reshape(P, RPP),
                "centers": centers,
            }
        )

    res = run_bass_kernel_spmd(
        nc, in_maps, core_ids=list(range(N_CORES)), trace=trace
    )

    total = np.float64(0.0)
    for r in res.results:
        total += np.sum(r["out"], dtype=np.float64)
    # masked-out entries: BS*(C_OUT-1) zeros, each clamped to 1e-12
    total += np.float64(BS) * np.float64(C_OUT - 1) * 1e-12
    loss = np.float32(total / BS)

    if trace:
        _CACHE["last_exec_time_ns"] = res.exec_time_ns
        _CACHE["last_results"] = res
    return np.array(loss, dtype=np.float32)
